# revision 61
# baseline (speedup 1.0000x reference)
"""NeuralMMU Trainium2 kernel (ACT-bound pipeline, variable chunks).

Per core: 131072 addrs = 256 blocks of 512, processed in chunks of
CHUNK_BLOCKS[t] blocks ([1,1,2] + [3]*84): small leading chunks for
pipeline fill, 1536-addr chunks in steady state to amortize the ACT
per-instruction overhead (222 cyc SBUF access latency).

  1. Host sends bit planes as bf16 (0/1), replicated x3 for the exact
     3-way bf16 split of W1 -> SBUF [96, 8192] per 16-block DMA group.
     Prologue pieces alternate the SP/ACT HWDGE queues so their fixed
     per-DMA overheads (650ns DGE + 900ns sem) pipeline during fill.
  2. L1 (PE): one bf16 matmul k=96 per 512-block: bits @ W1 splits
     -> PSUM hpre [128, chunk] f32 (exact f32).  Emitted two chunks
     ahead, BEFORE L2, which lets the coalesced PE semaphore-set that
     gates ACT(t+2) fire without waiting on L2(t)'s tiles.
  3. ACT Gelu(+b1): PSUM -> SBUF h f32.  Bottleneck engine:
     (chunk + 222) cyc @ 1.2 GHz per chunk; everything else overlaps.
  4. L2 transposed (PE): per 128-addr tile, h-slice [128,128] f32 is
     the STATIONARY and W2[:, :26] f32 the 26-col MOVING operand (fp32
     4 cyc/row applies to only 26 cols; exactness needs f32: logit
     threshold gaps go down to 2.5e-8) -> l2o [128 addr, NT*26] PSUM.
  5. DVE is_gt vs theta=(0.5-b2): bits bf16; DVE mult by 2^j (13-bit
     halves, all-bf16 for the fast DVE mode); DVE segmented reduce
     [128, 2NT, 13] -> [128, 2NT] f32 packed lo13/hi13 per addr tile.
  6. Grouped DMA (8 chunks, final group split) -> outp[128, 2048]
     col slices; host combines lo + 8192*hi.

PSUM: hpre 2 bufs x 3 banks + l2o 2 bufs x 1 bank = 8 banks (full).
Steady period = ACT busy 1465ns (back-to-back); overheads are fill,
drain, and fixed DMA/barrier latencies only.
Cost-model timeline: 135973 ns (baseline 299412; 2.20x).
"""

import numpy as np
from contextlib import ExitStack

import concourse.bass as bass
import concourse.mybir as mybir
import concourse.tile as tile
from concourse import bacc, bass_utils

B = 1_048_576
NCORES = 8
PER = B // NCORES          # 131072 addrs per core
BLK = 512
NBLKS = PER // BLK         # 256 blocks
CHUNK_BLOCKS = [1, 1, 2] + [3] * 84
assert sum(CHUNK_BLOCKS) == NBLKS
SOFF = np.cumsum([0] + CHUNK_BLOCKS).tolist()   # block offset per chunk
N_CHUNKS = len(CHUNK_BLOCKS)
MAXNB = max(CHUNK_BLOCKS)
GINB = 16                  # blocks per input DMA group
NGRP = NBLKS // GINB       # 16 groups
GOUT = 8                   # chunks per output DMA group
NBITS = 26
SEG = 13

F32 = mybir.dt.float32
BF16 = mybir.dt.bfloat16
AF = mybir.ActivationFunctionType
ALU = mybir.AluOpType
AX = mybir.AxisListType

# cst columns (f32): w1b bf16 [128,128] = 64 | w2e f32 26 | theta f32 26
# | pw2 bf16 [128,13] -> 7 (13 bf16 + pad) | b1 1
C_W1 = 0
C_W2 = 64
C_TH = 90
C_PW = 116
C_B1 = 123
NCST = 124


def build_nc() -> bass.Bass:
    nc = bacc.Bacc("TRN2")

    bp = nc.dram_tensor("bp", [NGRP, 96, GINB * BLK], BF16,
                        kind="ExternalInput")
    cst_d = nc.dram_tensor("cst", [128, NCST], F32, kind="ExternalInput")
    outp = nc.dram_tensor("outp", [128, 8 * NBLKS], F32,
                          kind="ExternalOutput")

    with ExitStack() as ctx:
        tc = ctx.enter_context(tile.TileContext(nc))
        const = ctx.enter_context(tc.tile_pool(name="const", bufs=1))
        rpool = ctx.enter_context(tc.tile_pool(name="rp", bufs=4))
        hp = ctx.enter_context(tc.tile_pool(name="hp", bufs=3))
        bitp = ctx.enter_context(tc.tile_pool(name="bitp", bufs=2))
        tmpp = ctx.enter_context(tc.tile_pool(name="tmpp", bufs=2))
        ocp = ctx.enter_context(tc.tile_pool(name="ocp", bufs=2))
        hprep = ctx.enter_context(tc.tile_pool(name="hprep", bufs=2,
                                               space="PSUM"))
        l2p = ctx.enter_context(tc.tile_pool(name="l2p", bufs=2, space="PSUM"))

        cst = const.tile([128, NCST], F32)
        w1b = cst[:, C_W1:C_W1 + 64].bitcast(BF16)   # [128,128]; rows 0-95
        w2e = cst[:, C_W2:C_W2 + NBITS]              # [128, 26] f32
        theta = cst[:, C_TH:C_TH + NBITS]            # [128, 26] f32
        pw2 = cst[:, C_PW:C_PW + 7].bitcast(BF16)[:, 0:SEG]   # [128, 13]
        b1c = cst[:, C_B1:C_B1 + 1]

        theta_b = {nb: theta.unsqueeze(1).broadcast_to([128, 4 * nb, NBITS])
                   for nb in set(CHUNK_BLOCKS)}
        pw2_b = {nb: pw2.unsqueeze(1).broadcast_to([128, 8 * nb, SEG])
                 for nb in set(CHUNK_BLOCKS)}

        Rtiles = {}

        def load_group(g, pieces=None):
            R = rpool.tile([96, GINB * BLK], BF16)
            Rtiles[g] = R
            old = [k for k in Rtiles if k < g - 3]
            for k in old:
                del Rtiles[k]
            for i, (b0, b1) in enumerate(pieces or [(0, GINB)]):
                # prologue pieces alternate SP/ACT HWDGE queues so their
                # fixed per-DMA overheads pipeline (ACT is idle then)
                eng = nc.scalar if (pieces and i % 2 == 1) else nc.sync
                eng.dma_start(
                    R[:, BLK * b0:BLK * b1], bp[g, :, BLK * b0:BLK * b1]
                )

        nc.sync.dma_start(cst[:], cst_d[:])
        # group 0 in pieces sized to feed the small leading chunks asap
        load_group(0, pieces=[(0, 2), (2, 4), (4, 7), (7, 10), (10, 16)])

        def l1mm(t):
            nb = CHUNK_BLOCKS[t]
            s = SOFF[t]
            glast = (s + nb - 1) // GINB
            for gpre in (glast + 1, glast + 2):
                if gpre < NGRP and gpre not in Rtiles:
                    load_group(gpre)
            hpre = hprep.tile([128, MAXNB * BLK], F32)
            for j in range(nb):
                b = s + j
                nc.tensor.matmul(
                    hpre[:, BLK * j:BLK * (j + 1)],
                    w1b[0:96, :],
                    Rtiles[b // GINB][0:96, BLK * (b % GINB):BLK * (b % GINB + 1)],
                    start=True, stop=True, tile_position=(0, 0),
                )
            return hpre

        hpres = {0: l1mm(0), 1: l1mm(1)}
        oc = None
        gofs = 0
        flushed = 0

        for t in range(N_CHUNKS):
            nb = CHUNK_BLOCKS[t]
            nt = 4 * nb                      # 128-addr tiles in this chunk
            chunk = nb * BLK

            h = hp.tile([128, MAXNB * BLK], F32)
            nc.scalar.activation(h[:, 0:chunk], hpres.pop(t)[:, 0:chunk],
                                 AF.Gelu, bias=b1c, scale=1.0)

            if t + 2 < N_CHUNKS:
                hpres[t + 2] = l1mm(t + 2)

            l2o = l2p.tile([128, 4 * MAXNB * NBITS], F32)
            for g in range(nt):
                nc.tensor.matmul(
                    l2o[:, NBITS * g:NBITS * (g + 1)],
                    h[:, 128 * g:128 * (g + 1)],
                    w2e[:],
                    start=True, stop=True, tile_position=(0, 0),
                )

            bits = bitp.tile([128, 4 * MAXNB * NBITS], BF16)
            nc.vector.tensor_tensor(
                bits[:, 0:nt * NBITS].rearrange("p (g b) -> p g b", b=NBITS),
                l2o[:, 0:nt * NBITS].rearrange("p (g b) -> p g b", b=NBITS),
                theta_b[nb],
                op=ALU.is_gt,
            )

            tmp = tmpp.tile([128, 4 * MAXNB * NBITS], BF16)
            nc.vector.tensor_tensor(
                tmp[:, 0:nt * NBITS].rearrange("p (s b) -> p s b", b=SEG),
                bits[:, 0:nt * NBITS].rearrange("p (s b) -> p s b", b=SEG),
                pw2_b[nb],
                op=ALU.mult,
            )

            if t % GOUT == 0:
                oc = ocp.tile([128, 8 * MAXNB * GOUT], F32)
                gofs = 8 * SOFF[t]
                flushed = 0
            o0 = 8 * SOFF[t] - gofs
            nc.vector.tensor_reduce(
                oc[:, o0:o0 + 2 * nt],
                tmp[:, 0:nt * NBITS].rearrange("p (s b) -> p s b", b=SEG),
                axis=AX.X, op=ALU.add,
            )
            if t % GOUT == GOUT - 1 or t >= N_CHUNKS - 2:
                nc.sync.dma_start(
                    outp[:, gofs + flushed:gofs + o0 + 2 * nt],
                    oc[:, flushed:o0 + 2 * nt],
                )
                flushed = o0 + 2 * nt

    return nc


def make_const_inputs(W1, b1, W2, b2):
    import ml_dtypes

    w1 = np.ascontiguousarray(W1[0:32, :], dtype=np.float32)
    hi = w1.astype(ml_dtypes.bfloat16)
    mid = (w1 - hi.astype(np.float32)).astype(ml_dtypes.bfloat16)
    lo = (w1 - hi.astype(np.float32) - mid.astype(np.float32)).astype(
        ml_dtypes.bfloat16
    )
    w1b = np.zeros((128, 128), dtype=ml_dtypes.bfloat16)
    w1b[0:32] = hi
    w1b[32:64] = mid
    w1b[64:96] = lo

    cst = np.zeros((128, NCST), dtype=np.float32)
    cst[:, C_W1:C_W1 + 64] = np.ascontiguousarray(w1b).view(np.float32)
    cst[:, C_W2:C_W2 + NBITS] = np.asarray(W2[:, :NBITS], dtype=np.float32)
    th = (0.5 - np.asarray(b2[:NBITS], dtype=np.float32))[None, :]
    cst[:, C_TH:C_TH + NBITS] = np.broadcast_to(th, (128, NBITS))
    pw = np.zeros((128, 14), dtype=ml_dtypes.bfloat16)
    pw[:, 0:SEG] = np.asarray([float(1 << i) for i in range(SEG)],
                              dtype=ml_dtypes.bfloat16)[None, :]
    cst[:, C_PW:C_PW + 7] = np.ascontiguousarray(pw).view(np.float32)
    cst[:, C_B1] = np.asarray(b1, dtype=np.float32)
    return {"cst": cst}


def make_bit_planes(virtual_addr):
    """Per-core [NGRP, 96, 8192] bf16 0/1 planes.

    Partition 32s + k (s = 0..2 replication) of group g, col m =
    bit k of addr (8192*g + m) within the core's address range.
    """
    import ml_dtypes

    va32 = np.asarray(virtual_addr).astype(np.uint32)
    ncores = va32.size // PER
    out = []
    for c in range(ncores):
        seg = va32[c * PER:(c + 1) * PER]
        byt = seg.view(np.uint8).reshape(NGRP, GINB * BLK, 4)
        bits = np.unpackbits(byt, axis=-1, bitorder="little")
        pl = bits.transpose(0, 2, 1)                  # [g, 32, 8192]
        pl3 = np.concatenate([pl, pl, pl], axis=1)    # [g, 96, 8192]
        out.append(np.ascontiguousarray(pl3).astype(ml_dtypes.bfloat16))
    return out


def combine_output(o):
    """[128, 2048] f32 -> [PER] int64.

    Column 2k / 2k+1 = lo13 / hi13 of global 128-addr tile k;
    addr = 128*k + partition.
    """
    lo = o[:, 0::2].astype(np.int64)      # [128, 1024]
    hi = o[:, 1::2].astype(np.int64)
    v = lo + 8192 * hi
    return v.T.reshape(-1)


_NC_CACHE = {}
TRACE = False
LAST_RES = None


def kernel(virtual_addr, W1, b1, W2, b2):
    global LAST_RES
    if "nc" not in _NC_CACHE:
        nc = build_nc()
        nc.finalize()
        _NC_CACHE["nc"] = nc
    nc = _NC_CACHE["nc"]

    consts = make_const_inputs(W1, b1, W2, b2)
    planes = make_bit_planes(virtual_addr)
    in_maps = [{"bp": planes[c], **consts} for c in range(NCORES)]

    res = bass_utils.run_bass_kernel_spmd(
        nc, in_maps, list(range(NCORES)), trace=TRACE
    )
    LAST_RES = res

    outs = [combine_output(res.results[c]["outp"]) for c in range(NCORES)]
    return np.concatenate(outs)


# revision 62
# speedup vs baseline: 1.0011x; 1.0011x over previous
"""NeuralMMU Trainium2 kernel (ACT-bound pipeline, variable chunks).

Per core: 131072 addrs = 256 blocks of 512, processed in chunks of
CHUNK_BLOCKS[t] blocks ([1,1,2] + [3]*84): small leading chunks for
pipeline fill, 1536-addr chunks in steady state to amortize the ACT
per-instruction overhead (222 cyc SBUF access latency).

  1. Host sends bit planes as bf16 (0/1), replicated x3 for the exact
     3-way bf16 split of W1 -> SBUF [96, 8192] per 16-block DMA group.
     Prologue pieces alternate the SP/ACT HWDGE queues so their fixed
     per-DMA overheads (650ns DGE + 900ns sem) pipeline during fill.
  2. L1 (PE): one bf16 matmul k=96 per 512-block: bits @ W1 splits
     -> PSUM hpre [128, chunk] f32 (exact f32).  Emitted two chunks
     ahead, BEFORE L2, which lets the coalesced PE semaphore-set that
     gates ACT(t+2) fire without waiting on L2(t)'s tiles.
  3. ACT Gelu(+b1): PSUM -> SBUF h f32.  Bottleneck engine:
     (chunk + 222) cyc @ 1.2 GHz per chunk; everything else overlaps.
  4. L2 transposed (PE): per 128-addr tile, h-slice [128,128] f32 is
     the STATIONARY and W2[:, :26] f32 the 26-col MOVING operand (fp32
     4 cyc/row applies to only 26 cols; exactness needs f32: logit
     threshold gaps go down to 2.5e-8) -> l2o [128 addr, NT*26] PSUM.
  5. DVE is_gt vs theta=(0.5-b2): bits bf16; DVE mult by 2^j (13-bit
     halves, all-bf16 for the fast DVE mode); DVE segmented reduce
     [128, 2NT, 13] -> [128, 2NT] f32 packed lo13/hi13 per addr tile.
  6. Grouped DMA (8 chunks, final group split) -> outp[128, 2048]
     col slices; host combines lo + 8192*hi.

PSUM: hpre 2 bufs x 3 banks + l2o 2 bufs x 1 bank = 8 banks (full).
Steady period = ACT busy 1465ns (back-to-back); overheads are fill,
drain, and fixed DMA/barrier latencies only.
Cost-model timeline: 135973 ns (baseline 299412; 2.20x).
"""

import numpy as np
from contextlib import ExitStack

import concourse.bass as bass
import concourse.mybir as mybir
import concourse.tile as tile
from concourse import bacc, bass_utils

B = 1_048_576
NCORES = 8
PER = B // NCORES          # 131072 addrs per core
BLK = 512
NBLKS = PER // BLK         # 256 blocks
CHUNK_BLOCKS = [1, 1, 2] + [3] * 84
assert sum(CHUNK_BLOCKS) == NBLKS
SOFF = np.cumsum([0] + CHUNK_BLOCKS).tolist()   # block offset per chunk
N_CHUNKS = len(CHUNK_BLOCKS)
MAXNB = max(CHUNK_BLOCKS)
GINB = 8                   # blocks per input DMA group
NGRP = NBLKS // GINB       # 16 groups
GOUT = 8                   # chunks per output DMA group
NBITS = 26
SEG = 13

F32 = mybir.dt.float32
BF16 = mybir.dt.bfloat16
AF = mybir.ActivationFunctionType
ALU = mybir.AluOpType
AX = mybir.AxisListType

# cst columns (f32): w1b bf16 [128,128] = 64 | w2e f32 26 | theta f32 26
# | pw2 bf16 [128,13] -> 7 (13 bf16 + pad) | b1 1
C_W1 = 0
C_W2 = 64
C_TH = 90
C_PW = 116
C_B1 = 123
NCST = 124


def build_nc() -> bass.Bass:
    nc = bacc.Bacc("TRN2")

    bp = nc.dram_tensor("bp", [NGRP, 96, GINB * BLK], BF16,
                        kind="ExternalInput")
    cst_d = nc.dram_tensor("cst", [128, NCST], F32, kind="ExternalInput")
    outp = nc.dram_tensor("outp", [128, 8 * NBLKS], F32,
                          kind="ExternalOutput")

    with ExitStack() as ctx:
        tc = ctx.enter_context(tile.TileContext(nc))
        const = ctx.enter_context(tc.tile_pool(name="const", bufs=1))
        rpool = ctx.enter_context(tc.tile_pool(name="rp", bufs=5))
        hp = ctx.enter_context(tc.tile_pool(name="hp", bufs=3))
        bitp = ctx.enter_context(tc.tile_pool(name="bitp", bufs=2))
        tmpp = ctx.enter_context(tc.tile_pool(name="tmpp", bufs=2))
        ocp = ctx.enter_context(tc.tile_pool(name="ocp", bufs=2))
        hprep = ctx.enter_context(tc.tile_pool(name="hprep", bufs=2,
                                               space="PSUM"))
        l2p = ctx.enter_context(tc.tile_pool(name="l2p", bufs=2, space="PSUM"))

        cst = const.tile([128, NCST], F32)
        w1b = cst[:, C_W1:C_W1 + 64].bitcast(BF16)   # [128,128]; rows 0-95
        w2e = cst[:, C_W2:C_W2 + NBITS]              # [128, 26] f32
        theta = cst[:, C_TH:C_TH + NBITS]            # [128, 26] f32
        pw2 = cst[:, C_PW:C_PW + 7].bitcast(BF16)[:, 0:SEG]   # [128, 13]
        b1c = cst[:, C_B1:C_B1 + 1]

        theta_b = {nb: theta.unsqueeze(1).broadcast_to([128, 4 * nb, NBITS])
                   for nb in set(CHUNK_BLOCKS)}
        pw2_b = {nb: pw2.unsqueeze(1).broadcast_to([128, 8 * nb, SEG])
                 for nb in set(CHUNK_BLOCKS)}

        Rtiles = {}

        def load_group(g, pieces=None):
            R = rpool.tile([96, GINB * BLK], BF16)
            Rtiles[g] = R
            old = [k for k in Rtiles if k < g - 3]
            for k in old:
                del Rtiles[k]
            for i, (b0, b1) in enumerate(pieces or [(0, GINB)]):
                # prologue pieces alternate SP/ACT HWDGE queues so their
                # fixed per-DMA overheads pipeline (ACT is idle then)
                eng = nc.scalar if (pieces and i % 2 == 1) else nc.sync
                eng.dma_start(
                    R[:, BLK * b0:BLK * b1], bp[g, :, BLK * b0:BLK * b1]
                )

        nc.sync.dma_start(cst[:], cst_d[:])
        # group 0 in pieces sized to feed the small leading chunks asap
        load_group(0, pieces=[(0, 2), (2, 4), (4, 6), (6, 8)])

        def l1mm(t):
            nb = CHUNK_BLOCKS[t]
            s = SOFF[t]
            glast = (s + nb - 1) // GINB
            for gpre in (glast + 1, glast + 2):
                if gpre < NGRP and gpre not in Rtiles:
                    load_group(gpre)
            hpre = hprep.tile([128, MAXNB * BLK], F32)
            for j in range(nb):
                b = s + j
                nc.tensor.matmul(
                    hpre[:, BLK * j:BLK * (j + 1)],
                    w1b[0:96, :],
                    Rtiles[b // GINB][0:96, BLK * (b % GINB):BLK * (b % GINB + 1)],
                    start=True, stop=True, tile_position=(0, 0),
                )
            return hpre

        hpres = {0: l1mm(0), 1: l1mm(1)}
        oc = None
        gofs = 0
        flushed = 0

        for t in range(N_CHUNKS):
            nb = CHUNK_BLOCKS[t]
            nt = 4 * nb                      # 128-addr tiles in this chunk
            chunk = nb * BLK

            h = hp.tile([128, MAXNB * BLK], F32)
            nc.scalar.activation(h[:, 0:chunk], hpres.pop(t)[:, 0:chunk],
                                 AF.Gelu, bias=b1c, scale=1.0)

            if t + 2 < N_CHUNKS:
                hpres[t + 2] = l1mm(t + 2)

            l2o = l2p.tile([128, 4 * MAXNB * NBITS], F32)
            for g in range(nt):
                nc.tensor.matmul(
                    l2o[:, NBITS * g:NBITS * (g + 1)],
                    h[:, 128 * g:128 * (g + 1)],
                    w2e[:],
                    start=True, stop=True, tile_position=(0, 0),
                )

            bits = bitp.tile([128, 4 * MAXNB * NBITS], BF16)
            nc.vector.tensor_tensor(
                bits[:, 0:nt * NBITS].rearrange("p (g b) -> p g b", b=NBITS),
                l2o[:, 0:nt * NBITS].rearrange("p (g b) -> p g b", b=NBITS),
                theta_b[nb],
                op=ALU.is_gt,
            )

            tmp = tmpp.tile([128, 4 * MAXNB * NBITS], BF16)
            nc.vector.tensor_tensor(
                tmp[:, 0:nt * NBITS].rearrange("p (s b) -> p s b", b=SEG),
                bits[:, 0:nt * NBITS].rearrange("p (s b) -> p s b", b=SEG),
                pw2_b[nb],
                op=ALU.mult,
            )

            if t % GOUT == 0:
                oc = ocp.tile([128, 8 * MAXNB * GOUT], F32)
                gofs = 8 * SOFF[t]
                flushed = 0
            o0 = 8 * SOFF[t] - gofs
            nc.vector.tensor_reduce(
                oc[:, o0:o0 + 2 * nt],
                tmp[:, 0:nt * NBITS].rearrange("p (s b) -> p s b", b=SEG),
                axis=AX.X, op=ALU.add,
            )
            if t % GOUT == GOUT - 1 or t >= N_CHUNKS - 2:
                nc.sync.dma_start(
                    outp[:, gofs + flushed:gofs + o0 + 2 * nt],
                    oc[:, flushed:o0 + 2 * nt],
                )
                flushed = o0 + 2 * nt

    return nc


def make_const_inputs(W1, b1, W2, b2):
    import ml_dtypes

    w1 = np.ascontiguousarray(W1[0:32, :], dtype=np.float32)
    hi = w1.astype(ml_dtypes.bfloat16)
    mid = (w1 - hi.astype(np.float32)).astype(ml_dtypes.bfloat16)
    lo = (w1 - hi.astype(np.float32) - mid.astype(np.float32)).astype(
        ml_dtypes.bfloat16
    )
    w1b = np.zeros((128, 128), dtype=ml_dtypes.bfloat16)
    w1b[0:32] = hi
    w1b[32:64] = mid
    w1b[64:96] = lo

    cst = np.zeros((128, NCST), dtype=np.float32)
    cst[:, C_W1:C_W1 + 64] = np.ascontiguousarray(w1b).view(np.float32)
    cst[:, C_W2:C_W2 + NBITS] = np.asarray(W2[:, :NBITS], dtype=np.float32)
    th = (0.5 - np.asarray(b2[:NBITS], dtype=np.float32))[None, :]
    cst[:, C_TH:C_TH + NBITS] = np.broadcast_to(th, (128, NBITS))
    pw = np.zeros((128, 14), dtype=ml_dtypes.bfloat16)
    pw[:, 0:SEG] = np.asarray([float(1 << i) for i in range(SEG)],
                              dtype=ml_dtypes.bfloat16)[None, :]
    cst[:, C_PW:C_PW + 7] = np.ascontiguousarray(pw).view(np.float32)
    cst[:, C_B1] = np.asarray(b1, dtype=np.float32)
    return {"cst": cst}


def make_bit_planes(virtual_addr):
    """Per-core [NGRP, 96, 8192] bf16 0/1 planes.

    Partition 32s + k (s = 0..2 replication) of group g, col m =
    bit k of addr (8192*g + m) within the core's address range.
    """
    import ml_dtypes

    va32 = np.asarray(virtual_addr).astype(np.uint32)
    ncores = va32.size // PER
    out = []
    for c in range(ncores):
        seg = va32[c * PER:(c + 1) * PER]
        byt = seg.view(np.uint8).reshape(NGRP, GINB * BLK, 4)
        bits = np.unpackbits(byt, axis=-1, bitorder="little")
        pl = bits.transpose(0, 2, 1)                  # [g, 32, 8192]
        pl3 = np.concatenate([pl, pl, pl], axis=1)    # [g, 96, 8192]
        out.append(np.ascontiguousarray(pl3).astype(ml_dtypes.bfloat16))
    return out


def combine_output(o):
    """[128, 2048] f32 -> [PER] int64.

    Column 2k / 2k+1 = lo13 / hi13 of global 128-addr tile k;
    addr = 128*k + partition.
    """
    lo = o[:, 0::2].astype(np.int64)      # [128, 1024]
    hi = o[:, 1::2].astype(np.int64)
    v = lo + 8192 * hi
    return v.T.reshape(-1)


_NC_CACHE = {}
TRACE = False
LAST_RES = None


def kernel(virtual_addr, W1, b1, W2, b2):
    global LAST_RES
    if "nc" not in _NC_CACHE:
        nc = build_nc()
        nc.finalize()
        _NC_CACHE["nc"] = nc
    nc = _NC_CACHE["nc"]

    consts = make_const_inputs(W1, b1, W2, b2)
    planes = make_bit_planes(virtual_addr)
    in_maps = [{"bp": planes[c], **consts} for c in range(NCORES)]

    res = bass_utils.run_bass_kernel_spmd(
        nc, in_maps, list(range(NCORES)), trace=TRACE
    )
    LAST_RES = res

    outs = [combine_output(res.results[c]["outp"]) for c in range(NCORES)]
    return np.concatenate(outs)
